# revision 43
# baseline (speedup 1.0000x reference)
"""Trainium2 Bass kernel for nn_Attention_67370857005350 (v2, fp8).

Dense transformer block:
  q  = relu(pw_q  @ relu(bn(dwconv3x3(x))))            (2,512,64,64)
  kv = relu(pw_kv @ relu(bn(dwconv3x3_s2(features))))  (2,1024,32,32)
  out = relu(w_out @ softmax(q.k/8).v + b_out)         (2,256,64,64)

Sharding: spatial over query pixels - core c handles batch c//4, query
rows 16*(c%4) .. +16 (1024 q pixels); kv branch duplicated per batch.
No cross-core communication.

v2 changes vs the 147us baseline:
  * fp8(e4m3) operands + MatmulPerfMode.DoubleRow (0.5 cyc/row) for the
    dw convs (diagonal matmuls), pointwise convs, q.k and attn.v.
  * q.k contracts d=64 as [32,2]-k-tile DR; attn.v contracts kv pairs
    as [128,2] DR (the officially-shaped variant).
  * N=1024 matmuls (PSUM 2-bank outputs) to halve instruction count.
  * softmax numerators stored as expm1 in fp8: k is pre-scaled by
    0.125/sqrt(2) so exp(dots*0.125) = exp(sqrt2*x); poly chunks store
    e' = (x+sqrt2)*x ~= expm1(sqrt2 x) whose small magnitude survives
    fp8; exp chunks (ACT) store full e. A per-head correction
    S_h = sum_kv v over poly-routed chunks (tiny PE matmuls) restores
    u = e@v, and the ones-column row-sum gets the same fix.
  * exp work split across ACT (native exp) / DVE / Pool (quadratic
    poly), routed per chunk-pair to balance engine load.
  * input DMAs consolidated into a handful of descriptors on the two
    HWDGE rings, biggest-blockers first.
"""

import os
import numpy as np

import concourse.bass as bass
import concourse.tile as tile
from concourse import bacc, mybir
from concourse.bass_utils import run_bass_kernel_spmd

# ---- problem constants (hardcoded; must match setup_inputs) ----
B = 2
DIM = 256            # input channels
INNER = 512          # q/k/v channels
HEADS = 8
D = INNER // HEADS   # 64 head dim
HW_ = 64             # image H = W
KVHW = 32            # kv image H = W after stride-2
NKV = KVHW * KVHW    # 1024 kv pixels per batch
N_CORES = 8
CORES_PER_BATCH = N_CORES // B
ROWS = HW_ // CORES_PER_BATCH   # 16 q rows per core
NQ = ROWS * HW_                 # 1024 q pixels per core
EPS = 1e-5
LN128 = float(np.log(128.0))    # exp bias: store 128*exp(dots/8) in fp8

FP = mybir.dt.float32
BF = mybir.dt.bfloat16
F8 = mybir.dt.float8e4

AF = mybir.ActivationFunctionType
OP = mybir.AluOpType
PM = mybir.MatmulPerfMode

# feature flags (HW-probed)
QK8 = os.environ.get("K_QK8", "1") == "1"      # q.k as fp8 DR32
# N=1024 matmul outputs are illegal (PSUM bank crossing); keep the
# half-split path unless proven otherwise.
N1024 = os.environ.get("K_N1024", "0") == "1"
# per-piece routing of the 64 softmax-numerator pieces (one piece = one
# [128,1024] exp over a kv chunk): A=ACT exp (stores 128*e in fp8),
# D=DVE poly, G=Pool poly; D/G start with a DVE copy that frees the
# PSUM dots early and store 128*expm1. Chunk-level mixing makes the two
# pieces of a pv pair drain on different engines concurrently (PSUM
# cadence), with counts tuned for ~equal engine busy time.
ROUTE = os.environ.get("K_ROUTE", "AGDGAAAA" * 8)


def _route(hp, j, P, s):
    return ROUTE[(hp * 2 + j) * 8 + P * 2 + s]


def _poly_chunks(h):
    hp, j = h // 2, h % 2
    return [(P, s) for P in range(4) for s in range(2)
            if _route(hp, j, P, s) != "A"]


def _mm_n(nc, out, lhsT, rhs, start, stop, perf_mode=None, n1024=None,
          tile_position=None):
    """matmul, split into N<=512 pieces unless N1024."""
    n = rhs.shape[-1]
    use1024 = N1024 if n1024 is None else n1024
    if n <= 512 or use1024:
        nc.tensor.matmul(out, lhsT, rhs, start=start, stop=stop,
                         perf_mode=perf_mode, tile_position=tile_position)
    else:
        assert n == 1024
        for half in range(2):
            sl = slice(half * 512, (half + 1) * 512)
            nc.tensor.matmul(out[..., sl], lhsT, rhs[..., sl],
                             start=start, stop=stop, perf_mode=perf_mode,
                             tile_position=tile_position)


def build_graph():
    nc = bacc.Bacc("TRN2", target_bir_lowering=False, debug=False,
                   enable_asserts=False)

    def din(name, shape, dt):
        return nc.dram_tensor(name, shape, dt, kind="ExternalInput").ap()

    # per-core shards (host pads/permutes/folds; see _prep_shards).
    # The v-path weights (dgk, wv) stay bf16: their fp8 quantization
    # noise is correlated across kv and survives attention averaging
    # (measured 0.9-1.6e-2 rel err); the q/k-path fp8 noise is flattened
    # by the tiny softmax exponent range (~9e-5).
    xq_d = din("xq", [64, 4 * 18 * 66], F8)    # [64, ct2, s2, 18, 66]
    fs_d = din("fs", [128, 2 * 66 * 66], BF)   # [128, ct2, 66, 66]
    dgq_d = din("dgq", [64, 4 * 9 * 128], F8)  # [64, ct2, s2, 9, 128]
    dgk_d = din("dgk", [128, 2 * 9 * 128], BF)  # [128, ct2, 9, 128]
    w8_d = din("w8", [128, 2048], F8)          # wq8|wk8
    w16_d = din("w16", [128, 2048], BF)        # wo16|wv16
    cb_d = din("cb32", [128, 10], FP)          # tqb2|tkb2|bout2|c16|zero|ln128
    out_d = nc.dram_tensor("out", [128, 2 * NQ], FP,
                           kind="ExternalOutput").ap()



    with tile.TileContext(nc) as tc:
        with (
            tc.tile_pool(name="const", bufs=1) as cpool,
            tc.tile_pool(name="inbuf", bufs=1) as inpool,
            tc.tile_pool(name="act", bufs=1) as actpool,
            tc.tile_pool(name="exp", bufs=6) as epool,
            tc.tile_pool(name="tp", bufs=3) as tpool,
            tc.tile_pool(name="small", bufs=2) as spool,
            tc.tile_pool(name="usb", bufs=2) as uspool,
            tc.tile_pool(name="ps", bufs=2, space="PSUM") as ps,
            tc.tile_pool(name="psu", bufs=2, space="PSUM") as psu,
        ):
            # ---------------- SBUF tiles ----------------
            xq = inpool.tile([64, 2, 2, 18, 66], F8)
            fs = inpool.tile([128, 2, 66, 66], BF)
            dgq = cpool.tile([64, 2, 2, 9, 128], F8)
            dgk = cpool.tile([128, 2, 9, 128], BF)
            w8 = cpool.tile([128, 2048], F8)
            wq8 = w8[:, 0:1024].rearrange("p (s m n) -> p s m n", s=2, m=4)
            wk8 = w8[:, 1024:2048].rearrange("p (s m n) -> p s m n", s=2, m=4)
            w16 = cpool.tile([128, 2048], BF)
            wo16 = w16[:, 0:1024].rearrange("p (k m n) -> p k m n", k=4, m=2)
            wv16 = w16[:, 1024:2048].rearrange("p (c n) -> p c n", c=2)
            cb = cpool.tile([128, 10], FP)

            # ---------------- input DMAs ----------------
            # scalar(ACT-HWDGE) ring: q-branch + weights, in gating order
            # sync(SP-HWDGE) ring: features image (kv branch), then output
            nc.scalar.dma_start(cb[:, :], cb_d)
            nc.scalar.dma_start(
                xq[:, :, :, :, :],
                xq_d.rearrange("p (c s a b) -> p c s a b", c=2, s=2, a=18))
            nc.scalar.dma_start(
                dgq[:, :, :, :, :],
                dgq_d.rearrange("p (c s t m) -> p c s t m", c=2, s=2, t=9))
            nc.scalar.dma_start(w8[:, :], w8_d)
            nc.sync.dma_start(
                dgk[:, :, :, :],
                dgk_d.rearrange("p (c t m) -> p c t m", c=2, t=9))
            fs_r = fs_d.rearrange("p (c a b) -> p c a b", c=2, a=66)
            for ct in range(2):
                nc.sync.dma_start(fs[:, ct, :, :], fs_r[:, ct])
            nc.sync.dma_start(w16[:, :], w16_d)

            tqb = cb[:, 0:2]    # per-ct dw-q bias columns
            tkb = cb[:, 2:4]
            boutc = cb[:, 4:6]  # per-mt to_out bias columns
            c16 = cb[:, 6:7]    # 16.0: poly (d+16)*d = 128*expm1(d/8)
            zc = cb[:, 7:8]     # zero column (AP scalar for relu max)
            ln128c = cb[:, 8:9]  # ln(128) exp bias

            # Pool's poly needs a full-width 16.0 tile (no TensorScalar
            # on Pool); ones8 holds 128.0 so S matches the 128x-scaled e.
            c16t = cpool.tile([128, 1024], BF)
            nc.gpsimd.memset(c16t[:, :], 16.0)
            ones8 = cpool.tile([128, 2, 32], F8)
            nc.gpsimd.memset(ones8[:, :, :], 128.0)

            tq = actpool.tile([128, 2, NQ], F8)     # ct = DR slot
            tkv = actpool.tile([128, 2, NKV], F8)   # for pw_k (DR)
            tkv16 = actpool.tile([128, 2, NKV], BF)  # for pw_v (bf16)
            # vt[p, P, h, s, m]: v^T staging, kv pair P slot s; m: v d
            # 0-63 | ones col 64 (row-sum trick) | zero pad to the legal
            # DR stationary width M=128 (M<128 non-pow2 crashes walrus)
            vt = actpool.tile([128, 4, HEADS, 2, 128], F8)
            nc.gpsimd.memset(vt[:, :, :, :, 64:65], 1.0)
            nc.gpsimd.memset(vt[:, :, :, :, 65:128], 0.0)
            if QK8:
                # q32/k32[p, hg, s, n]: head h at partitions 32*(h%4)..+32,
                # hg = h//4, s = d-half slot
                q32 = actpool.tile([128, 2, 2, NQ], F8)
                k32 = actpool.tile([128, 2, 2, NKV], F8)
            else:
                q_sb = actpool.tile([128, 4, NQ], BF)
                k_sb = actpool.tile([128, 4, NKV], BF)
            att = actpool.tile([128, 4, NQ], BF)
            s_sb = spool.tile([65, 8], FP, name="s_sb")
            osb = actpool.tile([128, 2, NQ], FP)

            # ---------------- depthwise convs on PE ----------------
            # q branch: fp8 DoubleRow diag matmuls ([64,2] channel tiles)
            # kv branch: bf16 diag matmuls (v-path precision)
            def dwq_conv(ct):
                acc = ps.tile([128, 1024], FP, tag="mm")
                for hf in range(2):
                    o = acc[:, hf * 512:(hf + 1) * 512]
                    for tap in range(9):
                        dy, dx = tap // 3, tap % 3
                        y0 = dy + hf * 8
                        rhs = xq[:, ct, :, y0:y0 + 8, dx:dx + 64]
                        nc.tensor.matmul(o, dgq[:, ct, :, tap, :], rhs,
                                         start=(tap == 0), stop=(tap == 8),
                                         perf_mode=PM.DoubleRow)
                nc.scalar.activation(tq[:, ct, :], acc[:, :], AF.Relu,
                                     bias=tqb[:, ct:ct + 1])

            def dwk_conv(ct):
                acc = ps.tile([128, 1024], FP, tag="mm")
                for hf in range(2):
                    o = acc[:, hf * 512:(hf + 1) * 512]
                    for tap in range(9):
                        dy, dx = tap // 3, tap % 3
                        y0 = dy + hf * 32
                        rhs = fs[:, ct, y0:y0 + 31:2, dx:dx + 63:2]
                        nc.tensor.matmul(o, dgk[:, ct, tap, :], rhs,
                                         start=(tap == 0), stop=(tap == 8))
                nc.scalar.activation(tkv16[:, ct, :], acc[:, :], AF.Relu,
                                     bias=tkb[:, ct:ct + 1])
                nc.scalar.activation(tkv[:, ct, :], acc[:, :], AF.Relu,
                                     bias=tkb[:, ct:ct + 1])

            for ct in range(2):
                dwq_conv(ct)
            for ct in range(2):
                dwk_conv(ct)

            # ---------------- pointwise convs (fp8 DR-128) -------------
            def pw_qk(wt, src, dst32, dst16, dve_epi=None):
                # column-parallel, out [ch 128, px] per mt; DR over in-ch.
                # With QK8 the host permutes the output channels so tile
                # mt covers exactly the (hg, s) = (mt//2, mt%2) slice of
                # the q32/k32 layout: one full-width epilogue per tile.
                for mt in range(4):
                    pq = ps.tile([128, 1024], FP, tag="mm")
                    _mm_n(nc, pq[:, :], wt[:, :, mt, :], src[:, :, :],
                          start=True, stop=True, perf_mode=PM.DoubleRow)
                    if QK8:
                        dst = dst32[:, mt // 2, mt % 2, :]
                    else:
                        dst = dst16[:, mt, :]
                    if dve_epi is not None:
                        nc.vector.tensor_scalar(dst, pq[:, :], zc[:, 0:1],
                                                None, op0=OP.max)
                    else:
                        nc.scalar.activation(dst, pq[:, :], AF.Relu)

            def pw_v():
                # row-parallel v^T (bf16): out [kv 128, vch 512] per chunk
                for i in range(4):
                    pv = ps.tile([128, 1024], FP, tag="mm")
                    for s in range(2):
                        c = 2 * i + s
                        for ct in range(2):
                            nc.tensor.matmul(
                                pv[:, 512 * s:512 * s + 512],
                                tkv16[:, ct, c * 128:(c + 1) * 128],
                                wv16[:, ct, :],
                                start=(ct == 0), stop=(ct == 1))
                    nc.vector.tensor_scalar(
                        vt[:, i, :, :, 0:64].rearrange("p h s d -> p s h d"),
                        pv[:, :].rearrange("p (s n) -> p s n", s=2),
                        zc[:, 0:1], None, op0=OP.max)

            if QK8:
                pw_qk(wq8, tq, q32, None)
                pw_qk(wk8, tkv, k32, None, dve_epi=True)
            else:
                pw_qk(wq8, tq, None, q_sb)
                pw_qk(wk8, tkv, None, k_sb)
            pw_v()

            # S_h = sum_kv v over poly-routed pairs (per-head correction
            # restoring the "+1" the expm1 chunks drop). Column h of one
            # [65, 1024] psum tile; copied to SBUF right away. Every head
            # must have >= 1 poly pair (the routing table guarantees it).
            nc.gpsimd.memset(s_sb[:, :], 0.0)
            if any(_poly_chunks(h) for h in range(HEADS)):
                s_ps = psu.tile([128, 1024], FP, tag="uR", name="s_ps")
                for h in range(HEADS):
                    pcs = _poly_chunks(h)
                    for j, (P, s) in enumerate(pcs):
                        nc.tensor.matmul(
                            s_ps[:, 32 * h:32 * h + 32],
                            vt[:, P, h, s, :], ones8[:, 0, :],
                            start=(j == 0), stop=(j == len(pcs) - 1))
                    if pcs:
                        nc.vector.tensor_copy(s_sb[:, h:h + 1],
                                              s_ps[0:65, 32 * h:32 * h + 1])

            # ---------------- attention ----------------
            def qk_mm(h, c, dp):
                if QK8:
                    p0 = 32 * (h % 4)
                    hg = h // 4
                    _mm_n(nc, dp[:, :],
                          k32[p0:p0 + 32, hg, :, c * 128:(c + 1) * 128],
                          q32[p0:p0 + 32, hg, :, :],
                          start=True, stop=True, perf_mode=PM.DoubleRow,
                          tile_position=(p0, 0))
                else:
                    p0 = 64 * (h % 2)
                    pt = h // 2
                    _mm_n(nc, dp[:, :],
                          k_sb[p0:p0 + 64, pt, c * 128:(c + 1) * 128],
                          q_sb[p0:p0 + 64, pt, :],
                          start=True, stop=True)

            def e_piece(route, dp, e, slot):
                # e8 pieces hold 128*softmax-numerator: A stores
                # 128*exp(d/8) via the ln(128) bias; D/G copy the dots
                # to SBUF first (frees the PSUM bank after one op) and
                # store (d+16)*d = 128*expm1(d/8) + O(d^3)
                if route == "A":
                    nc.scalar.activation(e[:, slot, :], dp[:, :], AF.Exp,
                                         scale=0.125, bias=ln128c[:, 0:1])
                elif route == "D":
                    xb = tpool.tile([128, 1024], BF, tag="xb")
                    nc.vector.tensor_copy(xb[:, :], dp[:, :])
                    nc.vector.scalar_tensor_tensor(
                        e[:, slot, :], xb[:, :], c16[:, 0:1], xb[:, :],
                        op0=OP.add, op1=OP.mult)
                else:
                    # Pool cannot read PSUM: DVE stages x, Pool does the poly
                    xb = tpool.tile([128, 1024], BF, tag="xb")
                    nc.vector.tensor_copy(xb[:, :], dp[:, :])
                    t = tpool.tile([128, 1024], BF, tag="tb")
                    nc.gpsimd.tensor_tensor(t[:, :], xb[:, :], c16t[:, :],
                                            op=OP.add)
                    nc.gpsimd.tensor_tensor(e[:, slot, :], t[:, :], xb[:, :],
                                            op=OP.mult)

            def normalize(h, uR, last=False):
                # copy u out of PSUM at once (u >= 0 so Relu == identity):
                # frees the uR bank for the next pair's pv instead of
                # holding it through this 4-engine latency chain
                kt = h // 2
                po = 64 * (h % 2)
                u_sb = uspool.tile([65, 1024], FP, tag="usb",
                                   name=f"usb_{h}")
                nc.scalar.activation(u_sb[:, :], uR[0:65, :], AF.Relu)
                rrow = spool.tile([1, 1024], FP, tag="rrow",
                                  name=f"rrow_{h}")
                nc.scalar.activation(rrow[:, :], u_sb[64:65, :], AF.Relu,
                                     bias=s_sb[64:65, h:h + 1])
                invr = spool.tile([1, 1024], FP, tag="invr",
                                  name=f"invr_{h}")
                nc.vector.reciprocal_approx_fast(invr[:, :], rrow[:, :])
                invrb = spool.tile([64, 1024], FP, tag="invrb",
                                   name=f"invrb_{h}")
                nc.gpsimd.partition_broadcast(invrb[:, :], invr[:, :])
                nc.vector.scalar_tensor_tensor(
                    att[po:po + 64, kt, :], u_sb[0:64, :],
                    s_sb[0:64, h:h + 1], invrb[:, :],
                    op0=OP.add, op1=OP.mult)

            def pv_mm(uR, P, h, e):
                _mm_n(nc, uR[:, :], vt[:, P, h, :, :],
                      e[:, :, :], start=(P == 0), stop=(P == 3),
                      perf_mode=PM.DoubleRow)

            for hp in range(4):
                heads = (2 * hp, 2 * hp + 1)
                uRs = [psu.tile([128, 1024], FP, tag="uR",
                                name=f"uR_{hp}_{j}") for j in range(2)]
                epend = {}
                for P in range(4):
                    for j, h in enumerate(heads):
                        e = epool.tile([128, 2, 1024], F8, tag="e",
                                       name=f"e_{hp}_{j}_{P}")
                        epend[(P, j)] = e
                        for s in range(2):
                            c = 2 * P + s
                            dp = ps.tile([128, 1024], FP, tag="mm",
                                         name=f"dp_{hp}_{j}_{c}")
                            qk_mm(h, c, dp)
                            e_piece(_route(hp, j, P, s), dp, e, s)
                        # pv lags one pair so the PE queue never blocks
                        # on an e tile still being produced
                        if P >= 1:
                            pv_mm(uRs[j], P - 1, h, epend.pop((P - 1, j)))
                for j, h in enumerate(heads):
                    pv_mm(uRs[j], 3, h, epend.pop((3, j)))
                    normalize(h, uRs[j])

            # ---------------- to_out + epilogue ----------------
            for mt in range(2):
                oso = ps.tile([128, 1024], FP, tag="mm", name=f"oso_{mt}")
                for kt in range(4):
                    _mm_n(nc, oso[:, :], wo16[:, kt, mt, :], att[:, kt, :],
                          start=(kt == 0), stop=(kt == 3))
                eng = nc.scalar if mt == 0 else None
                if eng is not None:
                    eng.activation(osb[:, mt, :], oso[:, :], AF.Relu,
                                   bias=boutc[:, mt:mt + 1])
                else:
                    nc.vector.tensor_scalar(osb[:, mt, :], oso[:, :],
                                            boutc[:, mt:mt + 1], 0.0,
                                            op0=OP.add, op1=OP.max)
                nc.sync.dma_start(
                    out_d.rearrange("p (m n) -> p m n", m=2)[:, mt],
                    osb[:, mt, :])

    nc.compile()
    return nc


_NC_CACHE = {}


def _get_nc():
    key = (QK8, N1024, ROUTE)
    if key not in _NC_CACHE:
        _NC_CACHE[key] = build_graph()
    return _NC_CACHE[key]


def _prep_shards(inputs):
    """Host-side sharding/layout prep. Returns in_maps for the 8 cores."""
    import ml_dtypes
    f8 = ml_dtypes.float8_e4m3
    bf = ml_dtypes.bfloat16
    f32 = lambda a: np.ascontiguousarray(np.asarray(a, np.float32))

    x = f32(inputs["x"])
    features = f32(inputs["features"])

    # fold BN into depthwise weights/bias
    sq = f32(inputs["bnq_g"]) / np.sqrt(f32(inputs["bnq_v"]) + EPS)
    sk = f32(inputs["bnk_g"]) / np.sqrt(f32(inputs["bnk_v"]) + EPS)
    dwq = (f32(inputs["dw_q"])[:, 0] * sq[:, None, None]).reshape(DIM, 9)
    dwk = (f32(inputs["dw_kv"])[:, 0] * sk[:, None, None]).reshape(DIM, 9)
    tqb = f32(inputs["bnq_b"]) - f32(inputs["bnq_m"]) * sq
    tkb = f32(inputs["bnk_b"]) - f32(inputs["bnk_m"]) * sk

    pw_q = f32(inputs["pw_q"])[:, :, 0, 0]       # (512, 256)
    pw_kv = f32(inputs["pw_kv"])[:, :, 0, 0]     # (1024, 256)
    w_out = f32(inputs["w_out"])[:, :, 0, 0]     # (256, 512)
    b_out = f32(inputs["b_out"])

    # diagonal tap matrices. q branch: DR tiles [64, ct, s, t, m] fp8;
    # kv branch: full-diag [128, ct, t, m] bf16 (v-path precision)
    dgq = np.zeros((64, 2, 2, 9, 128), np.float32)
    p = np.arange(64)
    for ct in range(2):
        for s in range(2):
            ch = ct * 128 + s * 64 + p
            dgq[p, ct, s, :, s * 64 + p] = dwq[ch]
    dgq = np.ascontiguousarray(dgq.reshape(64, -1).astype(f8))
    dgk = np.zeros((128, 2, 9, 128), np.float32)
    p2 = np.arange(128)
    for ct in range(2):
        dgk[p2, ct, :, p2] = dwk[ct * 128 + p2]
    dgk = np.ascontiguousarray(dgk.reshape(128, -1).astype(bf))

    # pw weights, DR over in-ch: w[p, s, mt, m] = W[perm(mt,m), s*128+p]
    def perm_qk(mt, m):
        # q32/k32 partition p holds head 4*hg + p//32, d-low p%32, with
        # d-half s in the free dim; tile mt = (hg, s) = (mt//2, mt%2)
        if not QK8:
            return mt * 128 + m
        h = 4 * (mt // 2) + m // 32
        d = (mt % 2) * 32 + m % 32
        return h * 64 + d

    wq8 = np.zeros((128, 2, 4, 128), np.float32)
    wk8 = np.zeros((128, 2, 4, 128), np.float32)
    for mt in range(4):
        for m in range(128):
            oc = perm_qk(mt, m)
            for s in range(2):
                wq8[:, s, mt, m] = pw_q[oc, s * 128:(s + 1) * 128]
                wk8[:, s, mt, m] = pw_kv[oc, s * 128:(s + 1) * 128]
    w8 = np.concatenate([wq8.reshape(128, -1), wk8.reshape(128, -1)],
                        axis=1).astype(f8)
    w8 = np.ascontiguousarray(w8)

    wo16 = np.zeros((128, 4, 2, 128), np.float32)
    for kt in range(4):
        for mt in range(2):
            wo16[:, kt, mt, :] = w_out[mt * 128:(mt + 1) * 128,
                                       kt * 128:(kt + 1) * 128].T
    wv16 = np.zeros((128, 2, 512), np.float32)
    for ct in range(2):
        wv16[:, ct, :] = pw_kv[INNER:, ct * 128:(ct + 1) * 128].T
    w16 = np.concatenate([wo16.reshape(128, -1), wv16.reshape(128, -1)],
                         axis=1).astype(bf)
    w16 = np.ascontiguousarray(w16)

    cb = np.zeros((128, 10), np.float32)
    cb[:, 0] = tqb[0:128]
    cb[:, 1] = tqb[128:256]
    cb[:, 2] = tkb[0:128]
    cb[:, 3] = tkb[128:256]
    cb[:, 4] = b_out[0:128]
    cb[:, 5] = b_out[128:256]
    cb[:, 6] = 16.0
    cb[:, 8] = LN128
    cb = np.ascontiguousarray(cb)

    # zero-padded images; xq in [64, ct, s, 18, 66] fp8 (DR channel
    # split), fs in [128, ct, 66, 66] bf16
    def img_split(img):  # img (DIM, 18, 66) padded slice
        h, w = img.shape[1], img.shape[2]
        o = np.zeros((64, 2, 2, h, w), np.float32)
        for ct in range(2):
            for s in range(2):
                o[:, ct, s] = img[ct * 128 + s * 64:ct * 128 + s * 64 + 64]
        return np.ascontiguousarray(o.reshape(64, -1).astype(f8))

    xpad = np.zeros((B, DIM, HW_ + 2, HW_ + 2), np.float32)
    xpad[:, :, 1:-1, 1:-1] = x
    fpad = np.zeros((B, DIM, HW_ + 2, HW_ + 2), np.float32)
    fpad[:, :, 1:-1, 1:-1] = features

    in_maps = []
    for c in range(N_CORES):
        b = c // CORES_PER_BATCH
        r0 = (c % CORES_PER_BATCH) * ROWS
        fs_b = np.ascontiguousarray(
            fpad[b].reshape(2, 128, 66, 66).transpose(1, 0, 2, 3)
            .reshape(128, -1).astype(bf))
        m = {
            "xq": img_split(xpad[b, :, r0:r0 + ROWS + 2, :]),
            "fs": fs_b,
            "dgq": dgq, "dgk": dgk, "w8": w8, "w16": w16, "cb32": cb,
        }
        in_maps.append(m)
    return in_maps


def kernel(**inputs):
    nc = _get_nc()
    in_maps = _prep_shards(inputs)
    trace = os.environ.get("KERNEL_TRACE", "0") == "1"
    res = run_bass_kernel_spmd(nc, in_maps, core_ids=list(range(N_CORES)),
                               trace=trace)
    if trace:
        kernel.last_exec_time_ns = res.exec_time_ns
        kernel.last_results = res
    out = np.zeros((B, DIM, HW_, HW_), np.float32)
    for c in range(N_CORES):
        b = c // CORES_PER_BATCH
        r0 = (c % CORES_PER_BATCH) * ROWS
        o = res.results[c]["out"].reshape(128, 2, ROWS, HW_)
        out[b, 0:128, r0:r0 + ROWS, :] = o[:, 0]
        out[b, 128:256, r0:r0 + ROWS, :] = o[:, 1]
    return out


if __name__ == "__main__":
    nc = build_graph()
    print("graph built + compiled OK")


# revision 44
# speedup vs baseline: 1.4917x; 1.4917x over previous
"""Trainium2 Bass kernel for nn_Attention_67370857005350.

Dense transformer block:
  q  = relu(pw_q  @ relu(bn(dwconv3x3(x))))            (2,512,64,64)
  kv = relu(pw_kv @ relu(bn(dwconv3x3_s2(features))))  (2,1024,32,32)
  out = relu(w_out @ softmax(q.k/8).v + b_out)         (2,256,64,64)

Sharding: spatial over query pixels — core c handles batch c//4, query
rows 16*(c%4) .. +16 (1024 q pixels).  Each core computes the full kv
branch for its batch (duplicated across the 4 cores of a batch; the kv
branch is ~12% of the FLOPs, and duplicating it removes every
collective).  No cross-core communication at all.

Per-core dataflow (all on-chip after the input DMAs):
  DVE:    3x3 depthwise convs as 9 scalar_tensor_tensor taps (q branch),
          relu epilogues, softmax normalize
  GPSIMD: kv-branch depthwise conv, partition-broadcast of 1/rowsum
  PE:     pointwise convs, q.k^T (transposed layout: kv on PSUM
          partitions), P@v via v^T produced directly by a row-parallel
          pointwise matmul (no PE transposes anywhere), to_out
  ACT:    exp (fused 1/8 scale), row-sum extraction copies

softmax is computed without the max-subtraction: dots = q.k/8 with
q,k >= 0 post-relu, and on this problem dots ∈ [0, 0.16], so exp is
safe in fp32 (softmax is shift-invariant so this matches the
reference's stabilized form).
"""

import os
import numpy as np

import concourse.bass as bass
import concourse.tile as tile
from concourse import bacc, mybir
from concourse.bass_utils import run_bass_kernel_spmd

# ---- problem constants (hardcoded; must match setup_inputs) ----
B = 2
DIM = 256            # input channels
INNER = 512          # q/k/v channels
HEADS = 8
D = INNER // HEADS   # 64 head dim
HW_ = 64             # image H = W
KVHW = 32            # kv image H = W after stride-2
NKV = KVHW * KVHW    # 1024 kv pixels per batch
N_CORES = 8
CORES_PER_BATCH = N_CORES // B
ROWS = HW_ // CORES_PER_BATCH   # 16 q rows per core
NQ = ROWS * HW_                 # 1024 q pixels per core
EPS = 1e-5
SCALE = float(D) ** -0.5        # 0.125

FP = mybir.dt.float32
FR = mybir.dt.float32r
BF = mybir.dt.bfloat16

# "f32r": fp32 storage, float32r matmuls (full-rate fp32-ish)
# "bf16": bf16 storage for matmul operands (weights pre-cast on host)
QUANT = os.environ.get("KERNEL_QUANT", "bf16")

AF = mybir.ActivationFunctionType
OP = mybir.AluOpType


def _mm(ap):
    return ap


def build_graph():
    """Build the SPMD graph (identical on all 8 cores)."""
    # dtype of matmul operands (DRAM weights / on-chip activations).
    # float32r is required end-to-end by the BIR verifier: producers of a
    # matmul operand must emit rounded-to-f32r values.
    w_dt = {"bf16": BF, "f32r": FR}.get(QUANT, FP)
    a_dt = w_dt

    nc = bacc.Bacc("TRN2", target_bir_lowering=False, debug=False,
                   enable_asserts=False)

    def din(name, shape, dt=FP):
        return nc.dram_tensor(name, shape, dt, kind="ExternalInput").ap()

    x_dt = BF if QUANT == "bf16" else FP  # host pre-casts images in bf16 mode
    # per-core shards (host pads/tranposes/folds; see kernel() below)
    xs = din("xs", [DIM, 18 * 66], x_dt)  # q-branch input rows, zero-padded
    fs = din("fs", [DIM, 66 * 66], x_dt)  # features (full batch), zero-padded
    dwq = din("dwq", [DIM, 9])            # BN-folded depthwise taps
    tqb = din("tqb", [DIM, 1])            # BN-folded bias
    dwk = din("dwk", [DIM, 9])
    tkb = din("tkb", [DIM, 1])
    pwqT = din("pwqT", [DIM, INNER], w_dt)    # lhsT for q pointwise
    pwkT = din("pwkT", [DIM, INNER], w_dt)    # lhsT for k pointwise
    wvT = din("wvT", [DIM, INNER], w_dt)      # rhs for v^T row-parallel pw
    woutT = din("woutT", [INNER, DIM], w_dt)  # lhsT for to_out
    bout = din("bout", [DIM, 1])
    if QUANT == "bf16":
        # per-(channel,tap) diagonal matrices for the q-branch depthwise
        # conv as PE matmuls (host-built; see _prep_shards)
        dgq = din("dgq", [DIM, 9 * 128], BF)
        dgq_r = dgq.rearrange("(t p) (k m) -> t p k m", p=128, k=9)
    out = nc.dram_tensor("out", [DIM, NQ], FP, kind="ExternalOutput").ap()

    xs_r = xs.rearrange("(t p) (a b) -> t p a b", p=128, a=18)
    fs_r = fs.rearrange("(t p) (a b) -> t p a b", p=128, a=66)
    dwq_r = dwq.rearrange("(t p) k -> t p k", p=128)
    dwk_r = dwk.rearrange("(t p) k -> t p k", p=128)
    tqb_r = tqb.rearrange("(t p) k -> t p k", p=128)
    tkb_r = tkb.rearrange("(t p) k -> t p k", p=128)
    pwqT_r = pwqT.rearrange("(t p) n -> t p n", p=128)
    pwkT_r = pwkT.rearrange("(t p) n -> t p n", p=128)
    wvT_r = wvT.rearrange("(t p) n -> t p n", p=128)
    woutT_r = woutT.rearrange("(t p) n -> t p n", p=128)
    bout_r = bout.rearrange("(t p) k -> t p k", p=128)
    out_r = out.rearrange("(t p) n -> t p n", p=128)

    with tile.TileContext(nc) as tc:
        with (
            tc.tile_pool(name="const", bufs=1) as cpool,
            tc.tile_pool(name="inbuf", bufs=1) as inpool,
            tc.tile_pool(name="acc", bufs=2) as accpool,
            tc.tile_pool(name="act", bufs=1) as actpool,
            tc.tile_pool(name="exp", bufs=4) as epool,
            tc.tile_pool(name="small", bufs=2) as spool,
            tc.tile_pool(name="usbp", bufs=3) as uspool,
            tc.tile_pool(name="ps", bufs=2, space="PSUM") as ps,
            tc.tile_pool(name="psu", bufs=2, space="PSUM") as psu,
        ):
            # ---------------- input DMAs ----------------
            # Three parallel DMA paths (SP-HWDGE, ACT-HWDGE, Pool-SWDGE),
            # ordered so the tensors that gate compute arrive first:
            #   sync:   dgq + x slice + q-branch weights  (PE dw-q matmuls)
            #   scalar: kv tap weights + features ct0     (DVE kv taps)
            #   gpsimd: features ct1 + remaining weights
            xp = inpool.tile([128, 2, 18, 66], x_dt)
            fp = inpool.tile([128, 2, 66, 66], x_dt)
            dwq_sb = cpool.tile([128, 2, 9], FP)
            dwk_sb = cpool.tile([128, 2, 9], FP)
            tqb_sb = cpool.tile([128, 2, 1], FP)
            tkb_sb = cpool.tile([128, 2, 1], FP)
            pwqT_sb = cpool.tile([128, 2, INNER], w_dt)
            pwkT_sb = cpool.tile([128, 2, INNER], w_dt)
            wvT_sb = cpool.tile([128, 2, INNER], w_dt)
            woutT_sb = cpool.tile([128, 4, DIM], w_dt)
            bout_sb = cpool.tile([128, 2, 1], FP)
            if QUANT == "bf16":
                dgq_sb = cpool.tile([128, 2, 9, 128], BF)
                nc.sync.dma_start(
                    dgq_sb[:, :, :, :],
                    dgq_r.rearrange("t p k m -> p t k m"))
            for t in range(2):
                nc.scalar.dma_start(dwk_sb[:, t, :], dwk_r[t])
                nc.scalar.dma_start(tkb_sb[:, t, :], tkb_r[t])
            nc.scalar.dma_start(fp[:, 0, :, :], fs_r[0])
            nc.gpsimd.dma_start(fp[:, 1, :, :], fs_r[1])
            nc.sync.dma_start(dwq_sb[:, :, :],
                              dwq_r.rearrange("t p k -> p t k"))
            nc.sync.dma_start(tqb_sb[:, :, :],
                              tqb_r.rearrange("t p k -> p t k"))
            nc.sync.dma_start(xp[:, :, :, :],
                              xs_r.rearrange("t p a b -> p t a b"))
            for t in range(2):
                nc.sync.dma_start(pwqT_sb[:, t, :], pwqT_r[t])
                nc.scalar.dma_start(pwkT_sb[:, t, :], pwkT_r[t])
                nc.gpsimd.dma_start(wvT_sb[:, t, :], wvT_r[t])
                nc.gpsimd.dma_start(bout_sb[:, t, :], bout_r[t])
            for t in range(4):
                nc.gpsimd.dma_start(woutT_sb[:, t, :], woutT_r[t])

            # v^T staging: [kv-chunk, head, 66] blocks; col 64 of each block
            # is the ones column (row-sum trick), col 65 unused padding.
            # (memset doesn't support f32r, so copy from an f32 ones tile.)
            vt_sb = actpool.tile([128, 8, HEADS, 66], a_dt)
            ones_sb = cpool.tile([128, 64], FP)
            nc.gpsimd.memset(ones_sb[:, :], 1.0)
            nc.vector.tensor_copy(
                vt_sb[:, :, :, 64:65],
                ones_sb[:, :].rearrange("p (a b c) -> p a b c", a=8, b=HEADS))

            tq = actpool.tile([128, 2, NQ], a_dt)
            tkv = actpool.tile([128, 2, NKV], a_dt)

            # ---------------- depthwise convs ----------------
            # All taps on DVE: GPSIMD's Pool ISA has no TensorScalarPtr
            # (per-partition scalar) op.  kv branch first — k/v gate more
            # PE work than q.
            def dw_conv(eng, src_ap, stride, n, wtile, btile, ct, dst,
                        half=None, epi_eng=None):
                # half: process only pixel rows [half] (kv branch) so the
                # first half of k/v unblocks attention chunks 0-3 early.
                acc = accpool.tile([128, n], FP, tag="dwacc")
                rows = 16 if stride == 1 else 16
                r0 = 0 if not half else (32 if stride == 1 else 32)
                av = acc[:, :].rearrange("p (a b) -> p a b", a=rows)
                for tap in range(9):
                    dy, dx = tap // 3, tap % 3
                    if stride == 1:
                        s = src_ap[:, ct, dy:dy + 16, dx:dx + 64]
                    else:
                        y0 = dy + half * 32
                        s = src_ap[:, ct, y0:y0 + 32:2, dx:dx + 64:2]
                    w = wtile[:, ct, tap:tap + 1]
                    if tap == 0:
                        eng.tensor_scalar(av, s, w, None, op0=OP.mult)
                    else:
                        eng.scalar_tensor_tensor(av, s, w, av,
                                                 op0=OP.mult, op1=OP.add)
                # t = relu(acc + bias); output dtype = a_dt
                if epi_eng is nc.scalar:
                    nc.scalar.activation(dst, acc[:, :], AF.Relu,
                                         bias=btile[:, ct, :])
                else:
                    nc.vector.tensor_scalar(dst, acc[:, :], btile[:, ct, :],
                                            0.0, op0=OP.add, op1=OP.max)

            def dwq_pe(ct):
                acc = psu.tile([128, 1024], FP, tag="uR")
                for half in range(2):
                    o = acc[:, half * 512:(half + 1) * 512]
                    for tap in range(9):
                        dy, dx = tap // 3, tap % 3
                        r0 = half * 8
                        rhs = xp[:, ct, dy + r0:dy + r0 + 8, dx:dx + 64]
                        nc.tensor.matmul(
                            o, dgq_sb[:, ct, tap, :], rhs,
                            start=(tap == 0), stop=(tap == 8))
                nc.scalar.activation(tq[:, ct, :], acc[:, :], AF.Relu,
                                     bias=tqb_sb[:, ct, :])

            q_sb = actpool.tile([128, 4, NQ], a_dt)
            k_sb = actpool.tile([128, 4, NKV], a_dt)

            def relu_epi(eng, out, in_):
                # relu from PSUM; on ScalarE (idle pre-attention, and relu
                # shares exp's ACT table set) or DVE (slack mid-attention)
                if eng is nc.scalar:
                    nc.scalar.activation(out, in_, AF.Relu)
                else:
                    eng.tensor_scalar(out, in_, 0.0, None, op0=OP.max)

            def pw_k_half(half, epi_eng):
                # k: [kc on partitions, kv pixels]  (column-parallel)
                for mt in range(4):
                    pk = ps.tile([128, 512], FP, tag="mm")
                    for ct in range(2):
                        nc.tensor.matmul(
                            pk[:, :],
                            _mm(pwkT_sb[:, ct, mt * 128:(mt + 1) * 128]),
                            _mm(tkv[:, ct, half * 512:(half + 1) * 512]),
                            start=(ct == 0), stop=(ct == 1))
                    relu_epi(epi_eng,
                             k_sb[:, mt, half * 512:(half + 1) * 512],
                             pk[:, :])

            def pw_v_range(kts, epi_eng):
                # v^T: [kv pixels on partitions, vc]  (row-parallel)
                for kt in kts:
                    pv = ps.tile([128, 1024], FP, tag="mm")
                    for ct in range(2):
                        nc.tensor.matmul(
                            pv[:, 0:512],
                            _mm(tkv[:, ct, kt * 128:(kt + 1) * 128]),
                            _mm(wvT_sb[:, ct, :]),
                            start=(ct == 0), stop=(ct == 1))
                    relu_epi(epi_eng, vt_sb[:, kt, :, 0:64],
                             pv[:, 0:512].rearrange("p (h d) -> p h d",
                                                    h=HEADS))

            def pw_q_all():
                # q: [qc on partitions, q pixels]
                for mt in range(4):
                    pq = ps.tile([128, 1024], FP, tag="mm")
                    for half in range(2):
                        o = pq[:, half * 512:(half + 1) * 512]
                        for ct in range(2):
                            nc.tensor.matmul(
                                o,
                                _mm(pwqT_sb[:, ct, mt * 128:(mt + 1) * 128]),
                                _mm(tq[:, ct, half * 512:(half + 1) * 512]),
                                start=(ct == 0), stop=(ct == 1))
                    relu_epi(nc.scalar, q_sb[:, mt, :], pq[:, :])

            # Emission order tuned for overlap: the DVE kv-tap chains are
            # the long serial pole at the start, so they lead; PE picks up
            # each downstream matmul group as its inputs land.
            for ct in range(2):
                dw_conv(nc.vector, fp, 2, NKV // 2, dwk_sb, tkb_sb, ct,
                        tkv[:, ct, 0:512], half=0, epi_eng=nc.scalar)
            if QUANT == "bf16":
                for ct in range(2):
                    dwq_pe(ct)
            else:
                for ct in range(2):
                    dw_conv(nc.vector, xp, 1, NQ, dwq_sb, tqb_sb, ct,
                            tq[:, ct, :], epi_eng=nc.scalar)
            pw_q_all()
            pw_k_half(0, nc.scalar)
            pw_v_range(range(0, 4), nc.scalar)
            # second tkv half: DVE taps run under the first attention pair;
            # their epilogues stay on DVE to keep ACT free for exp
            for ct in range(2):
                dw_conv(nc.vector, fp, 2, NKV // 2, dwk_sb, tkb_sb, ct,
                        tkv[:, ct, 512:1024], half=1, epi_eng=nc.vector)
            pw_k_half(1, nc.vector)
            pw_v_range(range(4, 8), nc.vector)

            # ---------------- attention ----------------
            # Heads processed in PAIRS with interleaved kv chunks: chunks
            # 0-3 of both heads only need the first tkv half, so they
            # overlap the DVE tap chains producing the second half.
            # After a head's P@v accumulation, uR is copied to SBUF at once
            # (frees its PSUM slot ~4us earlier than waiting for the whole
            # normalize chain), and to_out's K-accumulation is folded in
            # per pair (att rows of pair hp are exactly K-chunk hp).
            att_sb = actpool.tile([128, 4, NQ], a_dt)

            def normalize(h, u_sb, rrow):
                # att = u * (1/rowsum); rowsum = row 64 (ones-column trick).
                # (reciprocal_approx_fast only from partition 0 — reading it
                # at base partition 64 wedged the exec unit.)
                po = (h % 2) * 64
                pt = h // 2
                invr = spool.tile([1, 1024], FP, tag="invr")
                nc.vector.reciprocal_approx_fast(invr[:, :], rrow[:, :])
                invrb = spool.tile([64, 1024], FP, tag="invrb")
                nc.gpsimd.partition_broadcast(invrb[:, :], invr[:, :])
                nc.vector.tensor_tensor(att_sb[po:po + 64, pt, :],
                                        u_sb[0:64, :], invrb[:, :],
                                        op=OP.mult)

            for hp in range(HEADS // 2):
                heads = (2 * hp, 2 * hp + 1)
                uRs = [psu.tile([65, 1024], FP, tag="uR",
                                name=f"uR_{hp}_{j}") for j in range(2)]
                pend = [[], []]
                for c in range(8):
                    for j, h in enumerate(heads):
                        po = (h % 2) * 64
                        pt = h // 2
                        dp = ps.tile([128, 1024], FP, tag="mm")
                        for half in range(2):
                            nc.tensor.matmul(
                                dp[:, half * 512:(half + 1) * 512],
                                _mm(k_sb[po:po + 64, pt,
                                         c * 128:(c + 1) * 128]),
                                _mm(q_sb[po:po + 64, pt,
                                         half * 512:(half + 1) * 512]),
                                start=True, stop=True)
                        e = epool.tile([128, 1024], a_dt, tag="e")
                        nc.scalar.activation(e[:, :], dp[:, :], AF.Exp,
                                             scale=SCALE)
                        pend[j].append((c, e))
                        if len(pend[j]) > 1:
                            _emit_pv(nc, uRs[j], vt_sb, pend[j].pop(0), h)
                if hp < HEADS // 2 - 1:
                    for j, h in enumerate(heads):
                        _emit_pv(nc, uRs[j], vt_sb, pend[j].pop(0), h)
                        rrow = spool.tile([1, 1024], FP, tag="rrow",
                                          name=f"rrow_{hp}_{j}")
                        nc.vector.tensor_copy(rrow[:, :], uRs[j][64:65, :])
                        u_sb = uspool.tile([64, 1024], FP, tag="usb",
                                           name=f"usb_{hp}_{j}")
                        nc.vector.tensor_copy(u_sb[:, :], uRs[j][0:64, :])
                        normalize(h, u_sb, rrow)
                else:
                    # tail-optimized last pair: rrow extraction on ScalarE
                    # (idle after the final exp), then normalize multiplies
                    # interleaved with to_out column-half by column-half so
                    # the 16 to_out matmuls overlap the second-half mults.
                    invrbs = []
                    for j, h in enumerate(heads):
                        _emit_pv(nc, uRs[j], vt_sb, pend[j].pop(0), h)
                        rrow = spool.tile([1, 1024], FP, tag="rrow",
                                          name=f"rrowL_{j}")
                        nc.scalar.copy(rrow[:, :], uRs[j][64:65, :])
                        invr = spool.tile([1, 1024], FP, tag="invr",
                                          name=f"invrL_{j}")
                        nc.vector.reciprocal_approx_fast(invr[:, :],
                                                         rrow[:, :])
                        invrb = spool.tile([64, 1024], FP, tag="invrb",
                                           name=f"invrbL_{j}")
                        nc.gpsimd.partition_broadcast(invrb[:, :],
                                                      invr[:, :])
                        invrbs.append(invrb)
                    psos = []
                    for mt in range(2):
                        pso = ps.tile([128, 1024], FP, tag="mm",
                                      name=f"pso_{mt}")
                        psos.append(pso)
                    for half in range(2):
                        sl = slice(half * 512, (half + 1) * 512)
                        for j, h in enumerate(heads):
                            po = (h % 2) * 64
                            pt = h // 2
                            nc.vector.tensor_tensor(
                                att_sb[po:po + 64, pt, sl],
                                uRs[j][0:64, sl], invrbs[j][:, sl],
                                op=OP.mult)
                        for mt in range(2):
                            for ct in range(4):
                                nc.tensor.matmul(
                                    psos[mt][:, sl],
                                    _mm(woutT_sb[:, ct,
                                                 mt * 128:(mt + 1) * 128]),
                                    _mm(att_sb[:, ct, sl]),
                                    start=(ct == 0), stop=(ct == 3))

            # ---------------- output epilogue ----------------
            osb = actpool.tile([128, 2, NQ], FP)
            for mt in range(2):
                nc.vector.tensor_scalar(osb[:, mt, :], psos[mt][:, :],
                                        bout_sb[:, mt, :], 0.0,
                                        op0=OP.add, op1=OP.max)
                nc.sync.dma_start(out_r[mt], osb[:, mt, :])

    nc.compile()
    return nc


def _emit_pv(nc, uR, vt_sb, ce, h):
    c, e = ce
    for half in range(2):
        nc.tensor.matmul(uR[:, half * 512:(half + 1) * 512],
                         _mm(vt_sb[:, c, h, 0:65]),
                         _mm(e[:, half * 512:(half + 1) * 512]),
                         start=(c == 0), stop=(c == 7))


_NC_CACHE = {}


def _get_nc():
    key = QUANT
    if key not in _NC_CACHE:
        _NC_CACHE[key] = build_graph()
    return _NC_CACHE[key]


def _prep_shards(inputs):
    """Host-side sharding/layout prep. Returns in_maps for the 8 cores."""
    f32 = lambda a: np.ascontiguousarray(np.asarray(a, np.float32))
    w_np = np.float32 if QUANT != "bf16" else None

    def wcast(a):
        a = np.ascontiguousarray(np.asarray(a, np.float32))
        if QUANT == "bf16":
            import ml_dtypes
            a = a.astype(ml_dtypes.bfloat16)
        return a

    x = f32(inputs["x"])
    features = f32(inputs["features"])

    # fold BN into depthwise weights/bias
    sq = f32(inputs["bnq_g"]) / np.sqrt(f32(inputs["bnq_v"]) + EPS)
    sk = f32(inputs["bnk_g"]) / np.sqrt(f32(inputs["bnk_v"]) + EPS)
    dwq = f32(inputs["dw_q"])[:, 0] * sq[:, None, None]
    dwk = f32(inputs["dw_kv"])[:, 0] * sk[:, None, None]
    dwq = np.ascontiguousarray(dwq.reshape(DIM, 9))
    dwk = np.ascontiguousarray(dwk.reshape(DIM, 9))
    tqb = np.ascontiguousarray(
        (f32(inputs["bnq_b"]) - f32(inputs["bnq_m"]) * sq).reshape(DIM, 1))
    tkb = np.ascontiguousarray(
        (f32(inputs["bnk_b"]) - f32(inputs["bnk_m"]) * sk).reshape(DIM, 1))

    pw_q = f32(inputs["pw_q"])[:, :, 0, 0]       # (512, 256)
    pw_kv = f32(inputs["pw_kv"])[:, :, 0, 0]     # (1024, 256)
    w_out = f32(inputs["w_out"])[:, :, 0, 0]     # (256, 512)
    pwqT = wcast(pw_q.T)                          # (256, 512)
    pwkT = wcast(pw_kv[:INNER].T)                 # (256, 512)
    wvT = wcast(pw_kv[INNER:].T)                  # (256, 512)
    woutT = wcast(w_out.T)                        # (512, 256)
    bout = np.ascontiguousarray(f32(inputs["b_out"]).reshape(DIM, 1))

    dgq = None
    if QUANT == "bf16":
        # diagonal per-tap matrices for the PE q-branch depthwise conv
        import ml_dtypes
        d = np.zeros((DIM, 9, 128), np.float32)
        cc = np.arange(DIM)
        d[cc, :, cc % 128] = dwq
        dgq = np.ascontiguousarray(
            d.reshape(DIM, 9 * 128).astype(ml_dtypes.bfloat16))

    # zero-padded images
    xpad = np.zeros((B, DIM, HW_ + 2, HW_ + 2), np.float32)
    xpad[:, :, 1:-1, 1:-1] = x
    fpad = np.zeros((B, DIM, HW_ + 2, HW_ + 2), np.float32)
    fpad[:, :, 1:-1, 1:-1] = features

    if QUANT == "bf16":
        # images are stored/DMA'd in bf16 (the kernel computes in bf16)
        import ml_dtypes
        xpad = xpad.astype(ml_dtypes.bfloat16)
        fpad = fpad.astype(ml_dtypes.bfloat16)

    in_maps = []
    for c in range(N_CORES):
        b = c // CORES_PER_BATCH
        r0 = (c % CORES_PER_BATCH) * ROWS
        xs_c = np.ascontiguousarray(
            xpad[b, :, r0:r0 + ROWS + 2, :].reshape(DIM, 18 * 66))
        fs_c = np.ascontiguousarray(fpad[b].reshape(DIM, 66 * 66))
        m = {
            "xs": xs_c, "fs": fs_c,
            "dwq": dwq, "tqb": tqb, "dwk": dwk, "tkb": tkb,
            "pwqT": pwqT, "pwkT": pwkT, "wvT": wvT,
            "woutT": woutT, "bout": bout,
        }
        if dgq is not None:
            m["dgq"] = dgq
        in_maps.append(m)
    return in_maps


def kernel(**inputs):
    nc = _get_nc()
    in_maps = _prep_shards(inputs)
    trace = os.environ.get("KERNEL_TRACE", "0") == "1"
    res = run_bass_kernel_spmd(nc, in_maps, core_ids=list(range(N_CORES)),
                               trace=trace)
    if trace:
        kernel.last_exec_time_ns = res.exec_time_ns
        kernel.last_results = res
    out = np.zeros((B, DIM, HW_, HW_), np.float32)
    for c in range(N_CORES):
        b = c // CORES_PER_BATCH
        r0 = (c % CORES_PER_BATCH) * ROWS
        out[b, :, r0:r0 + ROWS, :] = res.results[c]["out"].reshape(
            DIM, ROWS, HW_)
    return out


if __name__ == "__main__":
    nc = build_graph()
    print("graph built + compiled OK")



# revision 45
# speedup vs baseline: 1.7903x; 1.2002x over previous
"""Trainium2 Bass kernel for nn_Attention_67370857005350.

Dense transformer block:
  q  = relu(pw_q  @ relu(bn(dwconv3x3(x))))            (2,512,64,64)
  kv = relu(pw_kv @ relu(bn(dwconv3x3_s2(features))))  (2,1024,32,32)
  out = relu(w_out @ softmax(q.k/8).v + b_out)         (2,256,64,64)

Sharding: spatial over query pixels — core c handles batch c//4, query
rows 16*(c%4) .. +16 (1024 q pixels).  Each core computes the full kv
branch for its batch (duplicated across the 4 cores of a batch; the kv
branch is ~12% of the FLOPs, and duplicating it removes every
collective).  No cross-core communication at all.

Per-core dataflow (all on-chip after the input DMAs):
  DVE:    3x3 depthwise convs as 9 scalar_tensor_tensor taps (q branch),
          relu epilogues, softmax normalize
  GPSIMD: kv-branch depthwise conv, partition-broadcast of 1/rowsum
  PE:     pointwise convs, q.k^T (transposed layout: kv on PSUM
          partitions), P@v via v^T produced directly by a row-parallel
          pointwise matmul (no PE transposes anywhere), to_out
  ACT:    exp (fused 1/8 scale), row-sum extraction copies

softmax is computed without the max-subtraction: dots = q.k/8 with
q,k >= 0 post-relu, and on this problem dots ∈ [0, 0.16], so exp is
safe in fp32 (softmax is shift-invariant so this matches the
reference's stabilized form).
"""

import os
import numpy as np

import concourse.bass as bass
import concourse.tile as tile
from concourse import bacc, mybir
from concourse.bass_utils import run_bass_kernel_spmd

# ---- problem constants (hardcoded; must match setup_inputs) ----
B = 2
DIM = 256            # input channels
INNER = 512          # q/k/v channels
HEADS = 8
D = INNER // HEADS   # 64 head dim
HW_ = 64             # image H = W
KVHW = 32            # kv image H = W after stride-2
NKV = KVHW * KVHW    # 1024 kv pixels per batch
N_CORES = 8
CORES_PER_BATCH = N_CORES // B
ROWS = HW_ // CORES_PER_BATCH   # 16 q rows per core
NQ = ROWS * HW_                 # 1024 q pixels per core
EPS = 1e-5
SCALE = float(D) ** -0.5        # 0.125

FP = mybir.dt.float32
FR = mybir.dt.float32r
BF = mybir.dt.bfloat16

# "f32r": fp32 storage, float32r matmuls (full-rate fp32-ish)
# "bf16": bf16 storage for matmul operands (weights pre-cast on host)
QUANT = os.environ.get("KERNEL_QUANT", "bf16")

AF = mybir.ActivationFunctionType
OP = mybir.AluOpType


def _mm(ap):
    return ap


def build_graph():
    """Build the SPMD graph (identical on all 8 cores)."""
    # dtype of matmul operands (DRAM weights / on-chip activations).
    # float32r is required end-to-end by the BIR verifier: producers of a
    # matmul operand must emit rounded-to-f32r values.
    w_dt = {"bf16": BF, "f32r": FR}.get(QUANT, FP)
    a_dt = w_dt

    nc = bacc.Bacc("TRN2", target_bir_lowering=False, debug=False,
                   enable_asserts=False)

    def din(name, shape, dt=FP):
        return nc.dram_tensor(name, shape, dt, kind="ExternalInput").ap()

    x_dt = BF if QUANT == "bf16" else FP  # host pre-casts images in bf16 mode
    # per-core shards (host pads/tranposes/folds; see kernel() below)
    xs = din("xs", [DIM, 18 * 66], x_dt)  # q-branch input rows, zero-padded
    fs = din("fs", [DIM, 66 * 66], x_dt)  # features (full batch), zero-padded
    # consolidated weight blobs (one DMA each instead of ~18 descriptors)
    b32_d = din("b32", [128, 42], FP)        # dwq|tqb|dwk|tkb|bout
    b16a_d = din("b16a", [128, 3328], BF)    # dgq|pwqT
    b16b_d = din("b16b", [128, 3072], BF)    # pwkT|wvT|woutT
    out = nc.dram_tensor("out", [DIM, NQ], FP, kind="ExternalOutput").ap()

    xs_r = xs.rearrange("(t p) (a b) -> t p a b", p=128, a=18)
    fs_r = fs.rearrange("(t p) (a b) -> t p a b", p=128, a=66)
    out_r = out.rearrange("(t p) n -> t p n", p=128)

    with tile.TileContext(nc) as tc:
        with (
            tc.tile_pool(name="const", bufs=1) as cpool,
            tc.tile_pool(name="inbuf", bufs=1) as inpool,
            tc.tile_pool(name="acc", bufs=2) as accpool,
            tc.tile_pool(name="act", bufs=1) as actpool,
            tc.tile_pool(name="exp", bufs=4) as epool,
            tc.tile_pool(name="small", bufs=2) as spool,
            tc.tile_pool(name="usbp", bufs=3) as uspool,
            tc.tile_pool(name="ps", bufs=2, space="PSUM") as ps,
            tc.tile_pool(name="psu", bufs=2, space="PSUM") as psu,
        ):
            # ---------------- input DMAs ----------------
            # Three parallel DMA paths (SP-HWDGE, ACT-HWDGE, Pool-SWDGE),
            # ordered so the tensors that gate compute arrive first:
            #   sync:   dgq + x slice + q-branch weights  (PE dw-q matmuls)
            #   scalar: kv tap weights + features ct0     (DVE kv taps)
            #   gpsimd: features ct1 + remaining weights
            xp = inpool.tile([128, 2, 18, 66], x_dt)
            fp = inpool.tile([128, 2, 66, 66], x_dt)
            b32 = cpool.tile([128, 42], FP)
            b16a = cpool.tile([128, 3328], BF)
            b16b = cpool.tile([128, 3072], BF)
            dwq_sb = b32[:, 0:18].rearrange("p (t k) -> p t k", t=2)
            tqb_sb = b32[:, 18:20].rearrange("p (t k) -> p t k", t=2)
            dwk_sb = b32[:, 20:38].rearrange("p (t k) -> p t k", t=2)
            tkb_sb = b32[:, 38:40].rearrange("p (t k) -> p t k", t=2)
            bout_sb = b32[:, 40:42].rearrange("p (t k) -> p t k", t=2)
            dgq_sb = b16a[:, 0:2304].rearrange("p (t k m) -> p t k m",
                                               t=2, k=9)
            pwqT_sb = b16a[:, 2304:3328].rearrange("p (t n) -> p t n", t=2)
            pwkT_sb = b16b[:, 0:1024].rearrange("p (t n) -> p t n", t=2)
            wvT_sb = b16b[:, 1024:2048].rearrange("p (t n) -> p t n", t=2)
            woutT_sb = b16b[:, 2048:3072].rearrange("p (t n) -> p t n", t=4)
            # sync ring: q-branch gaters; scalar: consts + features ct0;
            # gpsimd: features ct1 + k/v/out weights
            nc.scalar.dma_start(b32[:, :], b32_d)
            nc.sync.dma_start(xp[:, :, :, :],
                              xs_r.rearrange("t p a b -> p t a b"))
            nc.sync.dma_start(b16a[:, :], b16a_d)
            nc.scalar.dma_start(fp[:, 0, :, :], fs_r[0])
            nc.gpsimd.dma_start(fp[:, 1, :, :], fs_r[1])
            nc.gpsimd.dma_start(b16b[:, :], b16b_d)

            # v^T staging: [kv-chunk, head, 66] blocks; col 64 of each block
            # is the ones column (row-sum trick), col 65 unused padding.
            # (memset doesn't support f32r, so copy from an f32 ones tile.)
            vt_sb = actpool.tile([128, 8, HEADS, 66], a_dt)
            ones_sb = cpool.tile([128, 64], FP)
            nc.gpsimd.memset(ones_sb[:, :], 1.0)
            nc.vector.tensor_copy(
                vt_sb[:, :, :, 64:65],
                ones_sb[:, :].rearrange("p (a b c) -> p a b c", a=8, b=HEADS))

            tq = actpool.tile([128, 2, NQ], a_dt)
            tkv = actpool.tile([128, 2, NKV], a_dt)

            # ---------------- depthwise convs ----------------
            # All taps on DVE: GPSIMD's Pool ISA has no TensorScalarPtr
            # (per-partition scalar) op.  kv branch first — k/v gate more
            # PE work than q.
            def dw_conv(eng, src_ap, stride, n, wtile, btile, ct, dst,
                        half=None, epi_eng=None):
                # half: process only pixel rows [half] (kv branch) so the
                # first half of k/v unblocks attention chunks 0-3 early.
                acc = accpool.tile([128, n], FP, tag="dwacc")
                rows = 16 if stride == 1 else 16
                r0 = 0 if not half else (32 if stride == 1 else 32)
                av = acc[:, :].rearrange("p (a b) -> p a b", a=rows)
                for tap in range(9):
                    dy, dx = tap // 3, tap % 3
                    if stride == 1:
                        s = src_ap[:, ct, dy:dy + 16, dx:dx + 64]
                    else:
                        y0 = dy + half * 32
                        s = src_ap[:, ct, y0:y0 + 32:2, dx:dx + 64:2]
                    w = wtile[:, ct, tap:tap + 1]
                    if tap == 0:
                        eng.tensor_scalar(av, s, w, None, op0=OP.mult)
                    else:
                        eng.scalar_tensor_tensor(av, s, w, av,
                                                 op0=OP.mult, op1=OP.add)
                # t = relu(acc + bias); output dtype = a_dt
                if epi_eng is nc.scalar:
                    nc.scalar.activation(dst, acc[:, :], AF.Relu,
                                         bias=btile[:, ct, :])
                else:
                    nc.vector.tensor_scalar(dst, acc[:, :], btile[:, ct, :],
                                            0.0, op0=OP.add, op1=OP.max)

            def dwq_pe(ct):
                acc = psu.tile([128, 1024], FP, tag="uR")
                for half in range(2):
                    o = acc[:, half * 512:(half + 1) * 512]
                    for tap in range(9):
                        dy, dx = tap // 3, tap % 3
                        r0 = half * 8
                        rhs = xp[:, ct, dy + r0:dy + r0 + 8, dx:dx + 64]
                        nc.tensor.matmul(
                            o, dgq_sb[:, ct, tap, :], rhs,
                            start=(tap == 0), stop=(tap == 8))
                nc.scalar.activation(tq[:, ct, :], acc[:, :], AF.Relu,
                                     bias=tqb_sb[:, ct, :])

            q_sb = actpool.tile([128, 4, NQ], a_dt)
            k_sb = actpool.tile([128, 4, NKV], a_dt)

            def relu_epi(eng, out, in_):
                # relu from PSUM; on ScalarE (idle pre-attention, and relu
                # shares exp's ACT table set) or DVE (slack mid-attention)
                if eng is nc.scalar:
                    nc.scalar.activation(out, in_, AF.Relu)
                else:
                    eng.tensor_scalar(out, in_, 0.0, None, op0=OP.max)

            def pw_k_half(half, epi_eng):
                # k: [kc on partitions, kv pixels]  (column-parallel)
                for mt in range(4):
                    pk = ps.tile([128, 512], FP, tag="mm")
                    for ct in range(2):
                        nc.tensor.matmul(
                            pk[:, :],
                            _mm(pwkT_sb[:, ct, mt * 128:(mt + 1) * 128]),
                            _mm(tkv[:, ct, half * 512:(half + 1) * 512]),
                            start=(ct == 0), stop=(ct == 1))
                    relu_epi(epi_eng,
                             k_sb[:, mt, half * 512:(half + 1) * 512],
                             pk[:, :])

            def pw_v_range(kts, epi_eng):
                # v^T: [kv pixels on partitions, vc]  (row-parallel)
                for kt in kts:
                    pv = ps.tile([128, 1024], FP, tag="mm")
                    for ct in range(2):
                        nc.tensor.matmul(
                            pv[:, 0:512],
                            _mm(tkv[:, ct, kt * 128:(kt + 1) * 128]),
                            _mm(wvT_sb[:, ct, :]),
                            start=(ct == 0), stop=(ct == 1))
                    relu_epi(epi_eng, vt_sb[:, kt, :, 0:64],
                             pv[:, 0:512].rearrange("p (h d) -> p h d",
                                                    h=HEADS))

            def pw_q_all():
                # q: [qc on partitions, q pixels]
                for mt in range(4):
                    pq = ps.tile([128, 1024], FP, tag="mm")
                    for half in range(2):
                        o = pq[:, half * 512:(half + 1) * 512]
                        for ct in range(2):
                            nc.tensor.matmul(
                                o,
                                _mm(pwqT_sb[:, ct, mt * 128:(mt + 1) * 128]),
                                _mm(tq[:, ct, half * 512:(half + 1) * 512]),
                                start=(ct == 0), stop=(ct == 1))
                    relu_epi(nc.scalar, q_sb[:, mt, :], pq[:, :])

            # Emission order tuned for overlap: the DVE kv-tap chains are
            # the long serial pole at the start, so they lead; PE picks up
            # each downstream matmul group as its inputs land.
            for ct in range(2):
                dw_conv(nc.vector, fp, 2, NKV // 2, dwk_sb, tkb_sb, ct,
                        tkv[:, ct, 0:512], half=0, epi_eng=nc.scalar)
            if QUANT == "bf16":
                for ct in range(2):
                    dwq_pe(ct)
            else:
                for ct in range(2):
                    dw_conv(nc.vector, xp, 1, NQ, dwq_sb, tqb_sb, ct,
                            tq[:, ct, :], epi_eng=nc.scalar)
            pw_q_all()
            pw_k_half(0, nc.scalar)
            pw_v_range(range(0, 4), nc.scalar)
            # second tkv half: DVE taps run under the first attention pair;
            # their epilogues stay on DVE to keep ACT free for exp
            for ct in range(2):
                dw_conv(nc.vector, fp, 2, NKV // 2, dwk_sb, tkb_sb, ct,
                        tkv[:, ct, 512:1024], half=1, epi_eng=nc.vector)
            pw_k_half(1, nc.vector)
            pw_v_range(range(4, 8), nc.vector)

            # ---------------- attention ----------------
            # Heads processed in PAIRS with interleaved kv chunks: chunks
            # 0-3 of both heads only need the first tkv half, so they
            # overlap the DVE tap chains producing the second half.
            # After a head's P@v accumulation, uR is copied to SBUF at once
            # (frees its PSUM slot ~4us earlier than waiting for the whole
            # normalize chain), and to_out's K-accumulation is folded in
            # per pair (att rows of pair hp are exactly K-chunk hp).
            att_sb = actpool.tile([128, 4, NQ], a_dt)

            def normalize(h, u_sb, rrow):
                # att = u * (1/rowsum); rowsum = row 64 (ones-column trick).
                # (reciprocal_approx_fast only from partition 0 — reading it
                # at base partition 64 wedged the exec unit.)
                po = (h % 2) * 64
                pt = h // 2
                invr = spool.tile([1, 1024], FP, tag="invr")
                nc.vector.reciprocal_approx_fast(invr[:, :], rrow[:, :])
                invrb = spool.tile([64, 1024], FP, tag="invrb")
                nc.gpsimd.partition_broadcast(invrb[:, :], invr[:, :])
                nc.vector.tensor_tensor(att_sb[po:po + 64, pt, :],
                                        u_sb[0:64, :], invrb[:, :],
                                        op=OP.mult)

            for hp in range(HEADS // 2):
                heads = (2 * hp, 2 * hp + 1)
                uRs = [psu.tile([65, 1024], FP, tag="uR",
                                name=f"uR_{hp}_{j}") for j in range(2)]
                pend = [[], []]
                for c in range(8):
                    for j, h in enumerate(heads):
                        po = (h % 2) * 64
                        pt = h // 2
                        dp = ps.tile([128, 1024], FP, tag="mm")
                        for half in range(2):
                            nc.tensor.matmul(
                                dp[:, half * 512:(half + 1) * 512],
                                _mm(k_sb[po:po + 64, pt,
                                         c * 128:(c + 1) * 128]),
                                _mm(q_sb[po:po + 64, pt,
                                         half * 512:(half + 1) * 512]),
                                start=True, stop=True)
                        e = epool.tile([128, 1024], a_dt, tag="e")
                        nc.scalar.activation(e[:, :], dp[:, :], AF.Exp,
                                             scale=SCALE)
                        pend[j].append((c, e))
                        if len(pend[j]) > 1:
                            _emit_pv(nc, uRs[j], vt_sb, pend[j].pop(0), h)
                if hp < HEADS // 2 - 1:
                    for j, h in enumerate(heads):
                        _emit_pv(nc, uRs[j], vt_sb, pend[j].pop(0), h)
                        rrow = spool.tile([1, 1024], FP, tag="rrow",
                                          name=f"rrow_{hp}_{j}")
                        nc.vector.tensor_copy(rrow[:, :], uRs[j][64:65, :])
                        u_sb = uspool.tile([64, 1024], FP, tag="usb",
                                           name=f"usb_{hp}_{j}")
                        nc.vector.tensor_copy(u_sb[:, :], uRs[j][0:64, :])
                        normalize(h, u_sb, rrow)
                else:
                    # tail-optimized last pair: rrow extraction on ScalarE
                    # (idle after the final exp), then normalize multiplies
                    # interleaved with to_out column-half by column-half so
                    # the 16 to_out matmuls overlap the second-half mults.
                    invrbs = []
                    for j, h in enumerate(heads):
                        _emit_pv(nc, uRs[j], vt_sb, pend[j].pop(0), h)
                        rrow = spool.tile([1, 1024], FP, tag="rrow",
                                          name=f"rrowL_{j}")
                        nc.scalar.copy(rrow[:, :], uRs[j][64:65, :])
                        invr = spool.tile([1, 1024], FP, tag="invr",
                                          name=f"invrL_{j}")
                        nc.vector.reciprocal_approx_fast(invr[:, :],
                                                         rrow[:, :])
                        invrb = spool.tile([64, 1024], FP, tag="invrb",
                                           name=f"invrbL_{j}")
                        nc.gpsimd.partition_broadcast(invrb[:, :],
                                                      invr[:, :])
                        invrbs.append(invrb)
                    psos = []
                    for mt in range(2):
                        pso = ps.tile([128, 1024], FP, tag="mm",
                                      name=f"pso_{mt}")
                        psos.append(pso)
                    for half in range(2):
                        sl = slice(half * 512, (half + 1) * 512)
                        for j, h in enumerate(heads):
                            po = (h % 2) * 64
                            pt = h // 2
                            nc.vector.tensor_tensor(
                                att_sb[po:po + 64, pt, sl],
                                uRs[j][0:64, sl], invrbs[j][:, sl],
                                op=OP.mult)
                        for mt in range(2):
                            for ct in range(4):
                                nc.tensor.matmul(
                                    psos[mt][:, sl],
                                    _mm(woutT_sb[:, ct,
                                                 mt * 128:(mt + 1) * 128]),
                                    _mm(att_sb[:, ct, sl]),
                                    start=(ct == 0), stop=(ct == 3))

            # ---------------- output epilogue ----------------
            osb = actpool.tile([128, 2, NQ], FP)
            for mt in range(2):
                nc.vector.tensor_scalar(osb[:, mt, :], psos[mt][:, :],
                                        bout_sb[:, mt, :], 0.0,
                                        op0=OP.add, op1=OP.max)
                nc.sync.dma_start(out_r[mt], osb[:, mt, :])

    nc.compile()
    return nc


def _emit_pv(nc, uR, vt_sb, ce, h):
    c, e = ce
    for half in range(2):
        nc.tensor.matmul(uR[:, half * 512:(half + 1) * 512],
                         _mm(vt_sb[:, c, h, 0:65]),
                         _mm(e[:, half * 512:(half + 1) * 512]),
                         start=(c == 0), stop=(c == 7))


_NC_CACHE = {}


def _get_nc():
    key = QUANT
    if key not in _NC_CACHE:
        _NC_CACHE[key] = build_graph()
    return _NC_CACHE[key]


def _prep_shards(inputs):
    """Host-side sharding/layout prep. Returns in_maps for the 8 cores."""
    f32 = lambda a: np.ascontiguousarray(np.asarray(a, np.float32))
    w_np = np.float32 if QUANT != "bf16" else None

    def wcast(a):
        a = np.ascontiguousarray(np.asarray(a, np.float32))
        if QUANT == "bf16":
            import ml_dtypes
            a = a.astype(ml_dtypes.bfloat16)
        return a

    x = f32(inputs["x"])
    features = f32(inputs["features"])

    # fold BN into depthwise weights/bias
    sq = f32(inputs["bnq_g"]) / np.sqrt(f32(inputs["bnq_v"]) + EPS)
    sk = f32(inputs["bnk_g"]) / np.sqrt(f32(inputs["bnk_v"]) + EPS)
    dwq = f32(inputs["dw_q"])[:, 0] * sq[:, None, None]
    dwk = f32(inputs["dw_kv"])[:, 0] * sk[:, None, None]
    dwq = np.ascontiguousarray(dwq.reshape(DIM, 9))
    dwk = np.ascontiguousarray(dwk.reshape(DIM, 9))
    tqb = np.ascontiguousarray(
        (f32(inputs["bnq_b"]) - f32(inputs["bnq_m"]) * sq).reshape(DIM, 1))
    tkb = np.ascontiguousarray(
        (f32(inputs["bnk_b"]) - f32(inputs["bnk_m"]) * sk).reshape(DIM, 1))

    pw_q = f32(inputs["pw_q"])[:, :, 0, 0]       # (512, 256)
    pw_kv = f32(inputs["pw_kv"])[:, :, 0, 0]     # (1024, 256)
    w_out = f32(inputs["w_out"])[:, :, 0, 0]     # (256, 512)
    pwqT = wcast(pw_q.T)                          # (256, 512)
    pwkT = wcast(pw_kv[:INNER].T)                 # (256, 512)
    wvT = wcast(pw_kv[INNER:].T)                  # (256, 512)
    woutT = wcast(w_out.T)                        # (512, 256)
    bout = np.ascontiguousarray(f32(inputs["b_out"]).reshape(DIM, 1))

    dgq = None
    if QUANT == "bf16":
        # diagonal per-tap matrices for the PE q-branch depthwise conv
        import ml_dtypes
        d = np.zeros((DIM, 9, 128), np.float32)
        cc = np.arange(DIM)
        d[cc, :, cc % 128] = dwq
        dgq = np.ascontiguousarray(
            d.reshape(DIM, 9 * 128).astype(ml_dtypes.bfloat16))

    # zero-padded images
    xpad = np.zeros((B, DIM, HW_ + 2, HW_ + 2), np.float32)
    xpad[:, :, 1:-1, 1:-1] = x
    fpad = np.zeros((B, DIM, HW_ + 2, HW_ + 2), np.float32)
    fpad[:, :, 1:-1, 1:-1] = features

    if QUANT == "bf16":
        # images are stored/DMA'd in bf16 (the kernel computes in bf16)
        import ml_dtypes
        xpad = xpad.astype(ml_dtypes.bfloat16)
        fpad = fpad.astype(ml_dtypes.bfloat16)

    import ml_dtypes
    bfd = ml_dtypes.bfloat16

    def t2p(a, t):  # (t*128, k) -> [128, t*k] partition-major
        k = a.shape[-1] if a.ndim > 1 else 1
        return a.reshape(t, 128, k).transpose(1, 0, 2).reshape(128, t * k)

    b32_h = np.concatenate([
        t2p(dwq, 2), t2p(tqb, 2), t2p(dwk, 2), t2p(tkb, 2), t2p(bout, 2),
    ], axis=1).astype(np.float32)
    b32_h = np.ascontiguousarray(b32_h)
    b16a_h = np.concatenate([
        t2p(np.asarray(dgq, np.float32), 2),
        t2p(np.asarray(pwqT, np.float32), 2),
    ], axis=1).astype(bfd)
    b16a_h = np.ascontiguousarray(b16a_h)
    b16b_h = np.concatenate([
        t2p(np.asarray(pwkT, np.float32), 2),
        t2p(np.asarray(wvT, np.float32), 2),
        t2p(np.asarray(woutT, np.float32), 4),
    ], axis=1).astype(bfd)
    b16b_h = np.ascontiguousarray(b16b_h)

    in_maps = []
    for c in range(N_CORES):
        b = c // CORES_PER_BATCH
        r0 = (c % CORES_PER_BATCH) * ROWS
        xs_c = np.ascontiguousarray(
            xpad[b, :, r0:r0 + ROWS + 2, :].reshape(DIM, 18 * 66))
        fs_c = np.ascontiguousarray(fpad[b].reshape(DIM, 66 * 66))
        m = {"xs": xs_c, "fs": fs_c, "b32": b32_h, "b16a": b16a_h,
             "b16b": b16b_h}
        in_maps.append(m)
    return in_maps


def kernel(**inputs):
    nc = _get_nc()
    in_maps = _prep_shards(inputs)
    trace = os.environ.get("KERNEL_TRACE", "0") == "1"
    res = run_bass_kernel_spmd(nc, in_maps, core_ids=list(range(N_CORES)),
                               trace=trace)
    if trace:
        kernel.last_exec_time_ns = res.exec_time_ns
        kernel.last_results = res
    out = np.zeros((B, DIM, HW_, HW_), np.float32)
    for c in range(N_CORES):
        b = c // CORES_PER_BATCH
        r0 = (c % CORES_PER_BATCH) * ROWS
        out[b, :, r0:r0 + ROWS, :] = res.results[c]["out"].reshape(
            DIM, ROWS, HW_)
    return out


if __name__ == "__main__":
    nc = build_graph()
    print("graph built + compiled OK")

